# revision 34
# baseline (speedup 1.0000x reference)
"""DirGATConv on 8 Trainium2 NeuronCores (Bass/Tile), v2.

Problem: nn_DirGATConv  (N=50000 nodes, E=800000 edges, DIN=128, DOUT=64)
    out = 0.5 * GATConv(x, src->dst, W1) + 0.5 * GATConv(x, dst->src, W2)

Design (see git history for the one-hot-only baseline):
  * Nodes are grouped into 128-row chunks assigned to cores in contiguous
    ranges (49 chunk slots/core).  Conv1 groups edges by dst, conv2 by src;
    each core produces the output rows of its node range.  Zero collectives.
  * Phase A (replicated): one fused matmul per chunk computes
    [xw1 |0| a_s1 | a_d1 | xw2 |0| a_s2 | a_d2]; ACT-engine copies convert
    PSUM->f16 staging; tables stored row-permuted (row = (n%128)*G + n//128)
    so the staged writes are >=512B/descriptor.  Row = [xw(64) | 1 | a_s |
    junk].  Per-core a_d of own nodes is built on device from a per-core
    x^T slice: a_d row vectors (PE), broadcast to all partitions (PE ones-
    matmul) for the tail path, and in column layout for the diag path.
  * Phase B: per (chunk, conv) a single int16 index space addresses row
    PAIRS (idx = row>>1, elem 512B - same DMA cost as 256B).  The host
    assigns each dst's first q edges to "diagonal" slots (partition = dst
    local id) whose one-hot is the identity; remaining edges go to generic
    one-hot "tail" blocks.  A {0,-200} additive mask per (slot, half)
    selects the correct 256B half of each gathered pair and disables
    padding slots; it is folded into exp(leaky_relu(.)) computed 130-wide
    on the ACT engine.  Messages [w*xw | w] accumulate on the PE into
    PSUM [128, 65] per conv; column 64 is the softmax denominator.
"""

import math

import numpy as np

import concourse.bass as bass
import concourse.mybir as mybir
import concourse.tile as tile
from concourse import bacc, bass_utils
from concourse._compat import with_exitstack

# ---------------------------------------------------------------- constants
N = 50000
E = 800000
DIN = 128
DOUT = 64
ALPHA = 0.5
NEG_SLOPE = 0.2
NCORES = 8
P = 128

G = NCORES * math.ceil(math.ceil(N / P) / NCORES)  # 392 padded chunks
CPC = G // NCORES                                  # 49 chunk slots per core
NT = G * P                                         # 50176 padded node count
NPC = CPC * P                                      # 6272 nodes per core

WCOLS = 136        # wfull columns: 2 convs x [W(64) | 0 | a_s | a_d | pad]
RNDA = 3           # chunks per phase-A psum round (3*136*4B < 2KB bank)
STG = 12           # chunks per staging flush
SCN = 2            # chunks per gather super-tile
QCAP = 8           # max 128-slot blocks per gather call (1024-desc ring)
SCRATCH = 16384    # SWDGE descriptor carveout bytes (1024 descs);
                   # larger values fail to execute under this runtime
NQ = 4             # SWDGE queues, round-robin

f32 = mybir.dt.float32
f16 = mybir.dt.float16
i16 = mybir.dt.int16

VARIANT = "full"    # "full" | "gathers" | "phasea"  (perf-bisect variants)

_CACHE = {}


# ------------------------------------------------------------ host preprocess
def _prep_conv(key, gidx):
    """Edge layout for one conv.  key = group node (output row), gidx =
    gathered node.  Returns (q, kbt, per-core dict arrays)."""
    key = np.asarray(key, np.int64)
    gidx = np.asarray(gidx, np.int64)
    chunk = key // P
    dloc = key % P
    row = (gidx % P) * G + gidx // P
    pr = (row >> 1).astype(np.int16)
    parity = (row & 1).astype(np.int64)
    core = chunk // CPC
    slot = chunk % CPC

    D = np.bincount(chunk * P + dloc, minlength=G * P).reshape(
        NCORES, CPC, P)

    # tail blocks cost ~3x more vector-engine work than diag blocks, so
    # weight them when choosing the per-slot diag depth q.
    TAILW = 0.3
    q = np.zeros(CPC, np.int64)
    kbt = np.zeros(CPC, np.int64)
    for s in range(CPC):
        Ds = D[:, s, :]
        best = None
        for qq in range(0, int(Ds.max()) + 1):
            tails = np.maximum(Ds - qq, 0).sum(axis=1)
            kb = int(np.ceil(tails / P).max())
            cost = qq + (1.0 + TAILW) * kb
            if best is None or cost < best[0] or (
                    cost == best[0] and qq > best[1]):
                best = (cost, qq, kb)
        q[s], kbt[s] = best[1], best[2]

    nb = q + kbt                       # blocks per (slot, conv)
    boff = np.zeros(CPC + 1, np.int64)
    boff[1:] = np.cumsum(nb)
    tboff = np.zeros(CPC + 1, np.int64)
    tboff[1:] = np.cumsum(kbt)
    NB = int(boff[-1])
    KT = int(tboff[-1])

    # rank of each edge within its (chunk, dloc) group
    ck = chunk * P + dloc
    order = np.argsort(ck, kind="stable")
    counts = np.bincount(ck, minlength=G * P)
    gstart = np.zeros(G * P, np.int64)
    gstart[1:] = np.cumsum(counts)[:-1]
    rank = np.empty(len(ck), np.int64)
    rank[order] = np.arange(len(ck)) - gstart[ck[order]]

    qs_e = q[slot]
    isdiag = rank < qs_e

    # tail rank: position among tail edges of the same (core, slot)
    tkey = core * CPC + slot
    torder = np.argsort(np.where(isdiag, -1, tkey), kind="stable")
    tsorted = torder[int(isdiag.sum()):]          # tail edges, grouped
    tcounts = np.bincount(tkey[tsorted], minlength=NCORES * CPC)
    tstart = np.zeros(NCORES * CPC, np.int64)
    tstart[1:] = np.cumsum(tcounts)[:-1]
    trank = np.zeros(len(ck), np.int64)
    trank[tsorted] = np.arange(len(tsorted)) - tstart[tkey[tsorted]]

    blk = np.where(isdiag, rank, qs_e + trank // P)
    lane = np.where(isdiag, dloc, trank % P)
    gpos = (boff[slot] + blk) * P + lane

    TOT = NB * P
    ix = np.zeros((NCORES, TOT), np.int16)
    par = np.full((NCORES, TOT, 2), -200.0, np.float16)
    dlt = np.zeros((NCORES, P, max(KT, 1)), np.float16)

    ix[core, gpos] = pr
    par[core, gpos, parity] = 0.0
    tm = ~isdiag
    dlt[core[tm], lane[tm], tboff[slot[tm]] + trank[tm] // P] = \
        dloc[tm].astype(np.float16)

    # wrap indices for dma_gather: [128, TOT//16]
    ixw = ix.reshape(NCORES, TOT // 16, 16).transpose(0, 2, 1)
    ixw = np.ascontiguousarray(np.tile(ixw, (1, 8, 1)))
    # par device layout [128, NB, 2]
    parw = np.ascontiguousarray(
        par.reshape(NCORES, NB, P, 2).transpose(0, 2, 1, 3)
        .reshape(NCORES, P, NB * 2))
    return (tuple(int(v) for v in q), tuple(int(v) for v in kbt),
            ixw, parw, dlt)


def _preprocess(x, edge_index, W1, att_src1, att_dst1, b1,
                W2, att_src2, att_dst2, b2):
    src = np.asarray(edge_index[0], np.int64)
    dst = np.asarray(edge_index[1], np.int64)
    loops = np.arange(N, dtype=np.int64)
    all_src = np.concatenate([src, loops])
    all_dst = np.concatenate([dst, loops])

    q1, kbt1, ixw1, parw1, dlt1 = _prep_conv(all_dst, all_src)
    q2, kbt2, ixw2, parw2, dlt2 = _prep_conv(all_src, all_dst)

    xT = np.zeros((DIN, NT), np.float16)
    xT[:, :N] = np.asarray(x, np.float32).T.astype(np.float16)

    wfull = np.zeros((DIN, WCOLS), np.float64)
    for cv, (W, a_s, a_d) in enumerate((
            (W1, att_src1, att_dst1), (W2, att_src2, att_dst2))):
        o = cv * 68
        wfull[:, o:o + 64] = W
        wfull[:, o + 65] = W @ a_s
        wfull[:, o + 66] = W @ a_d
    wfull = wfull.astype(np.float16)
    adwt = np.stack([W1 @ att_dst1, W2 @ att_dst2], axis=1).astype(np.float16)

    iota = np.broadcast_to(np.arange(P, dtype=np.float16), (P, P)).copy()
    eye = np.eye(P, dtype=np.float16)
    onesrow = np.ones((1, P), np.float16)
    onescol = np.ones((P, 1), np.float16)
    bcomb = np.broadcast_to(
        ((1.0 - ALPHA) * np.asarray(b1, np.float64)
         + ALPHA * np.asarray(b2, np.float64)).astype(np.float32),
        (P, DOUT)).copy()

    common = dict(xT=xT, wfull=wfull, adwt=adwt, iota=iota, eye=eye,
                  onesrow=onesrow, onescol=onescol, bcomb=bcomb)
    per_core = []
    for k in range(NCORES):
        per_core.append(dict(
            xta=np.ascontiguousarray(xT[:, k * NPC:(k + 1) * NPC]),
            ix1=ixw1[k], ix2=ixw2[k],
            par1=parw1[k], par2=parw2[k],
            dlt1=dlt1[k], dlt2=dlt2[k]))
    meta = (q1, kbt1, q2, kbt2)
    return common, per_core, meta


# ------------------------------------------------------------- device program
@with_exitstack
def _emit(ctx, tc, outs, ins, meta):
    nc = tc.nc
    out_d = outs["out"]
    q1, kbt1, q2, kbt2 = meta
    qs_ = (q1, q2)
    kbt_ = (kbt1, kbt2)
    nb_ = tuple(tuple(a + b for a, b in zip(qs_[c], kbt_[c]))
                for c in range(2))
    boff_ = []
    tboff_ = []
    for c in range(2):
        bo = [0]
        to = [0]
        for s in range(CPC):
            bo.append(bo[-1] + nb_[c][s])
            to.append(to[-1] + kbt_[c][s])
        boff_.append(bo)
        tboff_.append(to)
    NBMAX = max(max(nb_[0]), max(nb_[1]))
    KTMAX = max(max(kbt_[0]), max(kbt_[1]), 1)

    t1_d = nc.dram_tensor("T1_tab", [NT, P], f16, kind="Internal").ap()
    t2_d = nc.dram_tensor("T2_tab", [NT, P], f16, kind="Internal").ap()
    t_views = [t.rearrange("(p g) c -> p g c", p=P) for t in (t1_d, t2_d)]
    t_pair = [t.rearrange("(r t) c -> r (t c)", t=2) for t in (t1_d, t2_d)]

    # persistent tiles used across both phases
    pre = ctx.enter_context(tc.tile_pool(name="pre", bufs=1))
    adbig = pre.tile([P, 2, CPC, P], f16)
    adcol = pre.tile([P, CPC, 2], f16)

    # ---------------- phase A: tables + per-core a_d ----------------
    with tc.tile_pool(name="pa0", bufs=1) as pa0, \
         tc.tile_pool(name="pa", bufs=2) as pa, \
         tc.tile_pool(name="pastg", bufs=2) as pastg, \
         tc.tile_pool(name="pap", bufs=2, space="PSUM") as pap, \
         tc.tile_pool(name="pad", bufs=2, space="PSUM") as pad:
        wf = pa0.tile([P, WCOLS], f16)
        nc.sync.dma_start(out=wf[:], in_=ins["wfull"][:])
        adwt = pa0.tile([P, 2], f16)
        nc.sync.dma_start(out=adwt[:], in_=ins["adwt"][:])
        ones1 = pa0.tile([1, P], f16)
        nc.sync.dma_start(out=ones1[:], in_=ins["onesrow"][:])

        for piece in range(NCORES):
            xt = pa.tile([P, NPC], f16, tag="xt")
            nc.sync.dma_start(
                out=xt[:], in_=ins["xT"][:, piece * NPC:(piece + 1) * NPC])
            stg = [pastg.tile([P, CPC, P], f16, tag=f"stg{t}",
                              name=f"stg{t}") for t in range(2)]
            gbase = piece * CPC
            for j0 in range(0, CPC, RNDA):
                r = min(RNDA, CPC - j0)
                ps = pap.tile([P, RNDA * WCOLS], f32, tag="pap")
                for k in range(r):
                    nc.tensor.matmul(
                        out=ps[:, k * WCOLS:(k + 1) * WCOLS],
                        lhsT=xt[:, (j0 + k) * P:(j0 + k + 1) * P],
                        rhs=wf[:], start=True, stop=True)
                psv = ps[:].rearrange("p (k c) -> p k c", k=RNDA)
                # T1 copy on ACT, T2 on DVE - balance the engines
                nc.scalar.copy(
                    out=stg[0][:, j0:j0 + r, 0:66], in_=psv[:, :r, 0:66])
                nc.vector.tensor_copy(
                    out=stg[1][:, j0:j0 + r, 0:66], in_=psv[:, :r, 68:134])
                for t in range(2):
                    nc.vector.memset(stg[t][:, j0:j0 + r, 64:65], 1.0)
            for t in range(2):
                nc.sync.dma_start(
                    out=t_views[t][:, gbase:gbase + CPC, :],
                    in_=stg[t][:, :, :])

        # ---- per-core a_d of own nodes ----
        xta = pa0.tile([P, NPC], f16)
        nc.sync.dma_start(out=xta[:], in_=ins["xta"][:])
        adrows = [pa0.tile([1, NPC], f16, tag=f"adrow{cv}",
                           name=f"adrow{cv}") for cv in range(2)]
        for cv in range(2):
            for j0 in range(0, NPC, 512):
                w = min(512, NPC - j0)
                aps = pad.tile([1, 512], f32, tag="adps")
                nc.tensor.matmul(out=aps[:, :w], lhsT=adwt[:, cv:cv + 1],
                                 rhs=xta[:, j0:j0 + w], start=True, stop=True)
                nc.scalar.copy(out=adrows[cv][:, j0:j0 + w], in_=aps[:, :w])
        for cv in range(2):
            for j0 in range(0, NPC, 512):
                w = min(512, NPC - j0)
                bps = pad.tile([P, 512], f32, tag="bps")
                nc.tensor.matmul(out=bps[:, :w], lhsT=ones1[:],
                                 rhs=adrows[cv][:, j0:j0 + w],
                                 start=True, stop=True)
                nc.scalar.copy(
                    out=adbig[:, cv, j0 // P:j0 // P + w // P, :],
                    in_=bps[:, :w].rearrange("p (k c) -> p k c", c=P))
        for c in range(CPC):
            cps = pad.tile([P, 2], f32, tag="cps")
            nc.tensor.matmul(out=cps[:], lhsT=xta[:, c * P:(c + 1) * P],
                             rhs=adwt[:], start=True, stop=True)
            nc.scalar.copy(out=adcol[:, c, :], in_=cps[:])

    # ---------------- phase B: edge aggregation ----------------
    with tc.tile_pool(name="pb0", bufs=1) as pb0, \
         tc.tile_pool(name="pg", bufs=2) as pg, \
         tc.tile_pool(name="pb", bufs=3) as pb, \
         tc.tile_pool(name="pbp", bufs=4, space="PSUM") as pbp:
        iota = pb0.tile([P, P], f16)
        nc.sync.dma_start(out=iota[:], in_=ins["iota"][:])
        eye = pb0.tile([P, P], f16)
        nc.sync.dma_start(out=eye[:], in_=ins["eye"][:])
        bcomb = pb0.tile([P, DOUT], f32)
        nc.sync.dma_start(out=bcomb[:], in_=ins["bcomb"][:])
        dlts = []
        pars = []
        for cv in range(2):
            kt = max(tboff_[cv][-1], 1)
            t = pb0.tile([P, kt], f16, tag=f"dlt{cv}", name=f"dlt{cv}")
            nc.sync.dma_start(out=t[:], in_=ins[f"dlt{cv + 1}"][:])
            dlts.append(t)
            nbt = boff_[cv][-1]
            t = pb0.tile([P, nbt, 2], f16, tag=f"par{cv}", name=f"par{cv}")
            nc.sync.dma_start(
                out=t[:].rearrange("p b t -> p (b t)"),
                in_=ins[f"par{cv + 1}"][:])
            pars.append(t)

        if VARIANT == "phasea":
            for s in range(CPC):
                nc.sync.dma_start(out=out_d[s * P:(s + 1) * P, :],
                                  in_=bcomb[:])
            return

        rrq = [0]
        NBS = [max(sum(nb_[cv][s0:s0 + SCN]) for s0 in range(0, CPC, SCN))
               for cv in range(2)]

        for s0 in range(0, CPC, SCN):
            scr = min(SCN, CPC - s0)
            gts = []
            for cv in range(2):
                nbsup = sum(nb_[cv][s0:s0 + scr])
                bo0 = boff_[cv][s0]
                gt = pg.tile([P, NBS[cv], 2, P], f16, tag=f"gt{cv}",
                             name=f"gt{cv}")
                ixt = pg.tile([P, NBS[cv] * 8], i16, tag=f"ixt{cv}",
                              name=f"ixt{cv}")
                nc.sync.dma_start(
                    out=ixt[:, :nbsup * 8],
                    in_=ins[f"ix{cv + 1}"][:, bo0 * 8:(bo0 + nbsup) * 8])
                gv = gt[:].rearrange("p k t c -> p k (t c)")
                for b0 in range(0, nbsup, QCAP):
                    bw = min(QCAP, nbsup - b0)
                    nc.gpsimd.dma_gather(
                        out_ap=gv[:, b0:b0 + bw, :], in_ap=t_pair[cv],
                        idxs_ap=ixt[:, b0 * 8:(b0 + bw) * 8],
                        num_idxs=bw * P, num_idxs_reg=bw * P,
                        elem_size=2 * P, queue_num=rrq[0])
                    rrq[0] = (rrq[0] + 1) % NQ
                gts.append(gt)
            if VARIANT == "gathers":
                for s in range(s0, s0 + scr):
                    nc.sync.dma_start(out=out_d[s * P:(s + 1) * P, :],
                                      in_=bcomb[:])
                continue
            for s in range(s0, s0 + scr):
                psums = []
                for cv in range(2):
                    qs = qs_[cv][s]
                    ks = kbt_[cv][s]
                    nb = nb_[cv][s]
                    bo = boff_[cv][s]
                    to = tboff_[cv][s]
                    go = boff_[cv][s] - boff_[cv][s0]
                    gt = gts[cv]
                    u = pb.tile([P, NBMAX, 2], f16, tag="u")
                    if qs:
                        nc.vector.scalar_tensor_tensor(
                            out=u[:, 0:qs, :], in0=gt[:, go:go + qs, :, 65],
                            scalar=adcol[:, s, cv:cv + 1],
                            in1=pars[cv][:, bo:bo + qs, :],
                            op0=mybir.AluOpType.add, op1=mybir.AluOpType.add)
                    oh = None
                    if ks:
                        oh = pb.tile([P, KTMAX, P], f16, tag="oh")
                        dlv = dlts[cv][:, to:to + ks]
                        nc.vector.tensor_tensor(
                            out=oh[:, :ks, :],
                            in0=dlv.unsqueeze(-1).to_broadcast([P, ks, P]),
                            in1=iota[:].unsqueeze(1).to_broadcast([P, ks, P]),
                            op=mybir.AluOpType.is_equal)
                        ohw = pb.tile([P, KTMAX, P], f16, tag="ohw")
                        nc.vector.tensor_tensor(
                            out=ohw[:, :ks, :], in0=oh[:, :ks, :],
                            in1=adbig[:, cv, s, :].unsqueeze(1)
                                .to_broadcast([P, ks, P]),
                            op=mybir.AluOpType.mult)
                        ade = pb.tile([P, KTMAX], f32, tag="ade")
                        nc.vector.tensor_reduce(
                            out=ade[:, :ks], in_=ohw[:, :ks, :],
                            axis=mybir.AxisListType.X, op=mybir.AluOpType.add)
                        nc.vector.tensor_tensor(
                            out=u[:, qs:nb, :],
                            in0=gt[:, go + qs:go + nb, :, 65],
                            in1=ade[:, :ks].unsqueeze(-1).to_broadcast(
                                [P, ks, 2]),
                            op=mybir.AluOpType.add)
                        nc.vector.tensor_tensor(
                            out=u[:, qs:nb, :], in0=u[:, qs:nb, :],
                            in1=pars[cv][:, bo + qs:bo + nb, :],
                            op=mybir.AluOpType.add)
                    ul = pb.tile([P, NBMAX, 2], f16, tag="ul")
                    nc.vector.scalar_tensor_tensor(
                        out=ul[:, :nb, :], in0=u[:, :nb, :], scalar=NEG_SLOPE,
                        in1=u[:, :nb, :],
                        op0=mybir.AluOpType.mult, op1=mybir.AluOpType.max)
                    wexp = pb.tile([P, NBMAX, 2, 65], f16, tag="wexp")
                    nc.scalar.activation(
                        out=wexp[:, :nb, :, :],
                        in_=ul[:, :nb, :].unsqueeze(-1).to_broadcast(
                            [P, nb, 2, 65]),
                        func=mybir.ActivationFunctionType.Exp)
                    msg = pb.tile([P, NBMAX, 2, 65], f16, tag="msg")
                    nc.vector.tensor_tensor(
                        out=msg[:, :nb, :, :], in0=wexp[:, :nb, :, :],
                        in1=gt[:, go:go + nb, :, 0:65],
                        op=mybir.AluOpType.mult)
                    ps = pbp.tile([P, 65], f32, tag=f"ps{cv}")
                    psums.append(ps)
                    for j in range(nb):
                        lhsT = eye[:] if j < qs else oh[:, j - qs, :]
                        for h in range(2):
                            nc.tensor.matmul(
                                out=ps[:], lhsT=lhsT, rhs=msg[:, j, h, :],
                                start=(j == 0 and h == 0),
                                stop=(j == nb - 1 and h == 1))
                # finalize
                os_ = []
                for cv in range(2):
                    ps = psums[cv]
                    den = pb.tile([P, 1], f32, tag="den")
                    nc.vector.tensor_scalar_max(den[:], ps[:, 64:65], 1e-30)
                    rec = pb.tile([P, 1], f32, tag="rec")
                    nc.vector.reciprocal(out=rec[:], in_=den[:])
                    rec2 = pb.tile([P, 1], f32, tag="rec2")
                    nc.vector.tensor_scalar_mul(
                        rec2[:], rec[:], (1.0 - ALPHA) if cv == 0 else ALPHA)
                    o = pb.tile([P, DOUT], f32, tag=f"o{cv}")
                    nc.scalar.mul(out=o[:], in_=ps[:, 0:64], mul=rec2[:])
                    os_.append(o)
                ofin = pb.tile([P, DOUT], f32, tag="ofin")
                nc.vector.tensor_tensor(
                    out=ofin[:], in0=os_[0][:], in1=os_[1][:],
                    op=mybir.AluOpType.add)
                nc.vector.tensor_tensor(
                    out=ofin[:], in0=ofin[:], in1=bcomb[:],
                    op=mybir.AluOpType.add)
                nc.sync.dma_start(out=out_d[s * P:(s + 1) * P, :],
                                  in_=ofin[:])


def _build(meta, stub=False):
    nc = bacc.Bacc("TRN2", target_bir_lowering=False, debug=False,
                   num_devices=NCORES, dynamic_dma_scratch_size=SCRATCH,
                   num_swdge_queues=NQ)
    q1, kbt1, q2, kbt2 = meta
    nb1 = sum(q1) + sum(kbt1)
    nb2 = sum(q2) + sum(kbt2)
    ins = {
        "xT": nc.dram_tensor("xT", [DIN, NT], f16, kind="ExternalInput").ap(),
        "xta": nc.dram_tensor("xta", [DIN, NPC], f16,
                              kind="ExternalInput").ap(),
        "wfull": nc.dram_tensor("wfull", [DIN, WCOLS], f16,
                                kind="ExternalInput").ap(),
        "adwt": nc.dram_tensor("adwt", [DIN, 2], f16,
                               kind="ExternalInput").ap(),
        "onesrow": nc.dram_tensor("onesrow", [1, P], f16,
                                  kind="ExternalInput").ap(),
        "onescol": nc.dram_tensor("onescol", [P, 1], f16,
                                  kind="ExternalInput").ap(),
        "iota": nc.dram_tensor("iota", [P, P], f16, kind="ExternalInput").ap(),
        "eye": nc.dram_tensor("eye", [P, P], f16, kind="ExternalInput").ap(),
        "bcomb": nc.dram_tensor("bcomb", [P, DOUT], f32,
                                kind="ExternalInput").ap(),
    }
    for cv, (nb, kbt) in enumerate(((nb1, kbt1), (nb2, kbt2))):
        kt = max(sum(kbt), 1)
        ins[f"ix{cv + 1}"] = nc.dram_tensor(
            f"ix{cv + 1}", [P, nb * 8], i16, kind="ExternalInput").ap()
        ins[f"par{cv + 1}"] = nc.dram_tensor(
            f"par{cv + 1}", [P, nb * 2], f16, kind="ExternalInput").ap()
        ins[f"dlt{cv + 1}"] = nc.dram_tensor(
            f"dlt{cv + 1}", [P, kt], f16, kind="ExternalInput").ap()
    outs = {"out": nc.dram_tensor("out", [NPC, DOUT], f32,
                                  kind="ExternalOutput").ap()}
    with tile.TileContext(nc) as tc:
        if stub:
            with tc.tile_pool(name="s", bufs=1) as p:
                t = p.tile([P, DOUT], f32)
                tc.nc.sync.dma_start(out=t[:], in_=ins["bcomb"][:])
                tc.nc.sync.dma_start(out=outs["out"][0:P, :], in_=t[:])
        else:
            _emit(tc, outs, ins, meta)
    nc.compile()
    return nc


# ------------------------------------------------------------------- entry
def kernel(x, edge_index, W1, att_src1, att_dst1, b1,
           W2, att_src2, att_dst2, b2):
    common, per_core, meta = _preprocess(
        np.asarray(x), np.asarray(edge_index),
        np.asarray(W1, np.float64), np.asarray(att_src1, np.float64),
        np.asarray(att_dst1, np.float64), np.asarray(b1, np.float32),
        np.asarray(W2, np.float64), np.asarray(att_src2, np.float64),
        np.asarray(att_dst2, np.float64), np.asarray(b2, np.float32))

    if meta not in _CACHE:
        _CACHE[meta] = _build(meta)
    nc = _CACHE[meta]

    in_maps = [dict(common, **pc) for pc in per_core]
    res = bass_utils.run_bass_kernel_spmd(
        nc, in_maps, core_ids=list(range(NCORES)))
    full = np.concatenate(
        [res.results[k]["out"] for k in range(NCORES)], axis=0)
    return np.ascontiguousarray(full[:N]).astype(np.float32)
